# revision 1
# baseline (speedup 1.0000x reference)
"""GroupedQueryAttention Trainium2 kernel (8 NeuronCores).

Sharding: core c -> (batch b=c//4, head-group g=c%4): q-heads [8g,8g+8),
kv-heads [2g,2g+2) of its batch. Flash-style causal attention in
transposed-score orientation (probsT[k,q]); denominators via a ones-column
appended to V. QK^T runs head-PAIRED on the PE (heads p / p+4 live in
partition halves, row-group tiling runs both matmuls concurrently). Exp is
split between ScalarE (activation) and VectorE (int16 Schraudolph exp:
bf16_bits = int16(score * A + B)). Per-q normalization uses a DRAM-roundtrip
reshape to [128,16] + reciprocal_approx_fast + gpsimd partition_broadcast.
Two half AllToAlls (pairs 01 / pairs 23) reshard head-split -> token-split,
overlapped with attention tail and o_proj head. o_proj streams wo per chunk
and accumulates two token-blocks per pass (8 PSUM banks).

Self-contained: hardcodes all shapes; only imports the concourse toolchain.
"""

import sys

for _p in ("/opt/trn_rl_repo", "/root/.axon_site/_ro/trn_rl_repo"):
    if _p not in sys.path:
        sys.path.insert(0, _p)

import math

import numpy as np
import ml_dtypes

import concourse.bass as bass
import concourse.mybir as mybir
import concourse.tile as tile
from concourse import bacc
from concourse.bass_utils import run_bass_kernel_spmd
from concourse.masks import make_identity

B, S, HID = 2, 2048, 2048
NH, NKV, HD = 32, 8, 64
GROUPS = NH // NKV
ROPE_BASE = 10000.0
NCORES = 8

BF = mybir.dt.bfloat16
F32 = mybir.dt.float32
I16 = mybir.dt.int16

NB = S // 128  # 16 k/q strip blocks
LQ = [S - 128 * j for j in range(NB)]
OFFX = [0]
for _j in range(NB):
    OFFX.append(OFFX[-1] + LQ[_j])
TOT = OFFX[-1]  # 17408

# strips whose exp runs on VectorE via int16 Schraudolph (rest on ScalarE)
VEC_STRIPS = frozenset((1, 3, 5, 7, 9, 13, 14, 15))
LOG2E = 1.4426950408889634
SCALE = 1.0 / math.sqrt(HD)
SCH_A = SCALE * LOG2E * 128.0
SCH_B = 127.0 * 128.0 - 5.6

_CACHED = {}


def _build_nc():
    nc = bacc.Bacc("TRN2", target_bir_lowering=False, debug=False,
                   num_devices=NCORES)

    hsT = nc.declare_dram_parameter("hsT", [17, 128, S], BF, isOutput=False)
    wq = nc.declare_dram_parameter("wq", [17, 128, 512], BF, isOutput=False)
    wk = nc.declare_dram_parameter("wk", [17, 128, 128], BF, isOutput=False)
    wv = nc.declare_dram_parameter("wv", [17, 128, 128], BF, isOutput=False)
    wo = nc.declare_dram_parameter("wo", [16, 128, HID], BF, isOutput=False)
    cos2 = nc.declare_dram_parameter("cos2", [128, S], BF, isOutput=False)
    sin2 = nc.declare_dram_parameter("sin2", [128, S], BF, isOutput=False)
    mb = nc.declare_dram_parameter("maskbin", [128, 128], BF, isOutput=False)
    bsel = nc.declare_dram_parameter("bsel", [128, 2], F32, isOutput=False)
    out_part = nc.declare_dram_parameter("out_part", [512, HID], F32,
                                         isOutput=True)

    MULT = mybir.AluOpType.mult
    ADD = mybir.AluOpType.add
    EXP = mybir.ActivationFunctionType.Exp

    with tile.TileContext(nc) as tc:
        with tc.tile_pool(name="pers", bufs=1) as pers, \
             tc.tile_pool(name="dram", bufs=1, space="DRAM") as dram:
            qT2 = pers.tile([128, 4, S], BF)
            kT2 = pers.tile([128, S], BF)
            v_aug = pers.tile([128, 2, NB, 65], BF)
            idn = pers.tile([128, 128], BF)
            make_identity(nc, idn)
            maskb = pers.tile([128, 128], BF)
            nc.sync.dma_start(out=maskb[:], in_=mb[:])
            bs = pers.tile([128, 2], F32)
            nc.sync.dma_start(out=bs[:], in_=bsel[:])
            nc.vector.memset(v_aug[:, :, :, 64:65], 1.0)
            # attg raw halves for A2A#1, prefetched during attention tail
            attg0A = pers.tile([128, 8, 512], BF)
            attg1A = pers.tile([128, 8, 512], BF)

            a2aA_in = dram.tile([NCORES, 256, 512], BF)
            a2aA_out = dram.tile([NCORES, 256, 512], BF)
            a2aB_in = dram.tile([NCORES, 256, 512], BF)
            a2aB_out = dram.tile([NCORES, 256, 512], BF)
            dd_dram = [dram.tile([1, S], BF, name=f"dd_{h}") for h in range(8)]
            rr_dram = [dram.tile([1, S], BF, name=f"rr_{h}") for h in range(8)]

            # ---------------- qkv projection + RoPE (bf16) ----------------
            with tc.tile_pool(name="proj", bufs=1) as pj, \
                 tc.tile_pool(name="projp", bufs=1, space="PSUM") as pjp:
                hsT_sb = pj.tile([128, 17, S], BF)
                wq_sb = pj.tile([128, 17, 512], BF)
                wk_sb = pj.tile([128, 17, 128], BF)
                wv_sb = pj.tile([128, 17, 128], BF)
                cos_sb = pj.tile([128, S], BF)
                nc.sync.dma_start(out=cos_sb[:], in_=cos2[:])
                sin_sb = pj.tile([128, S], BF)
                nc.sync.dma_start(out=sin_sb[:], in_=sin2[:])
                # grouped per-chunk loads so the ki=0 matmuls start early
                # while keeping the sync-queue trigger count low
                # interleave per-chunk loads so the ki=0 matmuls start early
                for ki in range(17):
                    nc.sync.dma_start(out=wq_sb[:, ki, :], in_=wq[ki])
                    nc.sync.dma_start(out=wk_sb[:, ki, :], in_=wk[ki])
                    nc.sync.dma_start(out=wv_sb[:, ki, :], in_=wv[ki])
                    nc.sync.dma_start(out=hsT_sb[:, ki, :], in_=hsT[ki])
                vT2 = pj.tile([128, S], BF)

                for t in range(6):  # 0-3: q pairs, 4: k pair, 5: v pair
                    accs = []
                    for Q in range(4):
                        acc = pjp.tile([128, 512], F32, tag="acc", bufs=5,
                                       name=f"acc_{t}_{Q}")
                        accs.append(acc)
                    for ki in range(17):
                        if t < 4:
                            lhsT = wq_sb[:, ki, 128 * t:128 * t + 128]
                        elif t == 4:
                            lhsT = wk_sb[:, ki, :]
                        else:
                            lhsT = wv_sb[:, ki, :]
                        for Q in range(4):
                            nc.tensor.matmul(
                                accs[Q][:], lhsT=lhsT,
                                rhs=hsT_sb[:, ki, 512 * Q:512 * Q + 512],
                                start=(ki == 0), stop=(ki == 16))
                    for Q in range(4):
                        sl = slice(512 * Q, 512 * Q + 512)
                        pq = accs[Q]
                        if t == 5:
                            nc.scalar.copy(vT2[:, sl], pq[:])
                            continue
                        tmp = pj.tile([128, 512], BF, tag="ropetmp", bufs=3,
                                      name=f"tmp_{t}_{Q}")
                        for (a, bb) in ((0, 32), (32, 0), (64, 96), (96, 64)):
                            nc.vector.tensor_tensor(
                                out=tmp[a:a + 32, :], in0=pq[bb:bb + 32, :],
                                in1=sin_sb[a:a + 32, sl], op=MULT)
                        tmp2 = pj.tile([128, 512], BF, tag="ropetmp2", bufs=3,
                                       name=f"tmp2_{t}_{Q}")
                        nc.vector.tensor_tensor(out=tmp2[:], in0=pq[:],
                                                in1=cos_sb[:, sl], op=MULT)
                        dst = qT2[:, t, sl] if t < 4 else kT2[:, sl]
                        nc.vector.tensor_tensor(out=dst, in0=tmp2[:],
                                                in1=tmp[:], op=ADD)

                # v: [2*64 dims, S] -> v_aug [kpos, kvslot, block, 65]
                for kb in range(NB):
                    pvt = pjp.tile([128, 128], BF, tag="vt", bufs=2,
                                   name=f"pvt_{kb}")
                    nc.tensor.transpose(pvt[:], vT2[:, 128 * kb:128 * kb + 128],
                                        idn[:])
                    nc.vector.tensor_copy(v_aug[:, 0, kb, 0:64], pvt[:, 0:64])
                    nc.vector.tensor_copy(v_aug[:, 1, kb, 0:64], pvt[:, 64:128])

            # ---------------- attention ----------------
            with tc.tile_pool(name="att", bufs=1) as at, \
                 tc.tile_pool(name="attp", bufs=1, space="PSUM") as atp:
                probs_of = {}

                def qk_strip(p, j, probsL, probsH):
                    q0 = 128 * j
                    L = LQ[j]
                    for cb in range(0, L, 1024):
                        w = min(1024, L - cb)
                        pss = []
                        for half, kb0 in ((0, 0), (1, 64)):
                            ps = atp.tile([128, 1024], F32, tag="sc", bufs=3,
                                          name=f"sc_{p}_{j}_{cb}_{half}")
                            pss.append(ps)
                        for m0 in (0, 512):
                            if m0 >= w:
                                continue
                            mw = min(512, w - m0)
                            for half, kb0 in ((0, 0), (1, 64)):
                                nc.tensor.matmul(
                                    pss[half][:, m0:m0 + mw],
                                    lhsT=kT2[kb0:kb0 + 64, q0:q0 + 128],
                                    rhs=qT2[kb0:kb0 + 64, p,
                                            q0 + cb + m0:q0 + cb + m0 + mw],
                                    start=True, stop=True)
                        for half, probs in ((0, probsL), (1, probsH)):
                            dst = probs[:, OFFX[j] + cb:OFFX[j] + cb + w]
                            src = pss[half][:, 0:w]
                            if j in VEC_STRIPS:
                                nc.vector.tensor_scalar(
                                    out=dst.bitcast(I16), in0=src,
                                    scalar1=SCH_A, scalar2=SCH_B,
                                    op0=MULT, op1=ADD)
                            else:
                                nc.scalar.activation(dst, src, EXP,
                                                     scale=SCALE)
                    # causal mask on the diagonal block (after cb=0 exp)
                    for probs in (probsL, probsH):
                        nc.vector.tensor_tensor(
                            out=probs[:, OFFX[j]:OFFX[j] + 128],
                            in0=probs[:, OFFX[j]:OFFX[j] + 128],
                            in1=maskb[:], op=MULT)

                def pv_chunk(p, half, c, aa):
                    # head = p (half 0) or p+4 (half 1); kv slot = half
                    probs = probs_of[(p, half)]
                    pvt = atp.tile([65, 512], F32, tag="pv", bufs=2,
                                   name=f"pv_{p}_{half}_{c}")
                    for j in range(4 * c + 4):
                        if j <= 4 * c:
                            col = OFFX[j] + 512 * c - 128 * j
                            nc.tensor.matmul(
                                pvt[:, 0:512], lhsT=v_aug[:, half, j, :],
                                rhs=probs[:, col:col + 512],
                                start=(j == 0), stop=(j == 4 * c + 3))
                        else:
                            d0 = 128 * (j - 4 * c)
                            nc.tensor.matmul(
                                pvt[:, d0:512], lhsT=v_aug[:, half, j, :],
                                rhs=probs[:, OFFX[j]:OFFX[j] + 512 - d0],
                                start=False, stop=(j == 4 * c + 3))
                    nc.vector.tensor_copy(aa[:, 512 * c:512 * c + 512], pvt[:])

                def normalize_and_send(p, half, aa):
                    h = p + 4 * half
                    # den row (partition 64) -> DRAM -> [128,16] -> recip ->
                    # DRAM -> [1,S] row -> broadcast -> multiply
                    nc.gpsimd.dma_start(out=dd_dram[h][:], in_=aa[64:65, :])
                    den_rs = at.tile([128, 16], BF, tag="denrs", bufs=2,
                                     name=f"denrs_{h}")
                    nc.gpsimd.dma_start(
                        out=den_rs[:],
                        in_=dd_dram[h][:].rearrange("1 (p c) -> p c", p=128))
                    den_f = at.tile([128, 16], F32, tag="denf", bufs=2,
                                    name=f"denf_{h}")
                    nc.vector.tensor_copy(den_f[:], den_rs[:])
                    rec_f = at.tile([128, 16], F32, tag="recf", bufs=2,
                                    name=f"recf_{h}")
                    nc.vector.reciprocal_approx_fast(out=rec_f[:], in_=den_f[:])
                    rec_b = at.tile([128, 16], BF, tag="recb", bufs=2,
                                    name=f"recb_{h}")
                    nc.vector.tensor_copy(rec_b[:], rec_f[:])
                    nc.gpsimd.dma_start(
                        out=rr_dram[h][:].rearrange("1 (p c) -> p c", p=128),
                        in_=rec_b[:])
                    rec_row = at.tile([1, S], BF, tag="recrow", bufs=1,
                                      name=f"recrow_{h}")
                    nc.gpsimd.dma_start(out=rec_row[:], in_=rr_dram[h][:])
                    rb = at.tile([64, S], BF, tag="rb", bufs=1, name=f"rb_{h}")
                    nc.gpsimd.partition_broadcast(rb[:], rec_row[:])
                    attn_n = at.tile([64, S], BF, tag="attn", bufs=1,
                                     name=f"attn_{h}")
                    nc.vector.tensor_tensor(out=attn_n[:], in0=aa[0:64, :],
                                            in1=rb[:], op=MULT)
                    a2a_in = a2aA_in if p < 2 else a2aB_in
                    rowoff = 128 * (p % 2) + 64 * half
                    for r in range(NCORES):
                        nc.sync.dma_start(
                            out=a2a_in[r, rowoff:rowoff + 64, :],
                            in_=attn_n[:, 512 * (r % 4):512 * (r % 4) + 512])

                def pv_pair_unit(p, unit):
                    # unit 0..7: head half = unit//4, chunk c = unit%4
                    half, c = unit // 4, unit % 4
                    key = (p, half, "aa")
                    if key not in probs_of:
                        probs_of[key] = at.tile([65, S], BF, tag="aaug",
                                                bufs=4, name=f"aa_{p}_{half}")
                    aa = probs_of[key]
                    pv_chunk(p, half, c, aa)
                    if c == 3:
                        normalize_and_send(p, half, aa)

                for p in range(4):
                    probsL = at.tile([128, TOT], BF, tag="probsL", bufs=2,
                                     name=f"probsL_{p}")
                    probsH = at.tile([128, TOT], BF, tag="probsH", bufs=2,
                                     name=f"probsH_{p}")
                    probs_of[(p, 0)] = probsL
                    probs_of[(p, 1)] = probsH
                    for j in range(NB):
                        qk_strip(p, j, probsL, probsH)
                        if p >= 1 and j % 2 == 1:
                            pv_pair_unit(p - 1, j // 2)
                        if p == 2 and j == 15:
                            # pairs 0,1 fully sent -> first half A2A
                            nc.gpsimd.collective_compute(
                                "AllToAll", mybir.AluOpType.bypass,
                                replica_groups=[list(range(NCORES))],
                                ins=[a2aA_in.opt()], outs=[a2aA_out.opt()])
                            nc.scalar.dma_start(
                                out=attg0A[:],
                                in_=a2aA_out[0:4].rearrange(
                                    "s (c p) n -> p (s c) n", p=128))
                            nc.scalar.dma_start(
                                out=attg1A[:],
                                in_=a2aA_out[4:8].rearrange(
                                    "s (c p) n -> p (s c) n", p=128))
                        if p == 3 and j % 4 == 3:
                            # own PV interleaved: chunk c for both heads
                            c = j // 4
                            pv_pair_unit(3, c)
                            pv_pair_unit(3, 4 + c)
                nc.gpsimd.collective_compute(
                    "AllToAll", mybir.AluOpType.bypass,
                    replica_groups=[list(range(NCORES))],
                    ins=[a2aB_in.opt()], outs=[a2aB_out.opt()])

            # ---------------- o_proj (my 512 tokens, all 2048 od) ---------
            with tc.tile_pool(name="op", bufs=1) as po, \
                 tc.tile_pool(name="opp", bufs=1, space="PSUM") as pop:
                wo_sb = po.tile([128, 16, HID], BF)
                for c4 in range(0, 16, 4):
                    nc.scalar.dma_start(
                        out=wo_sb[:, c4:c4 + 4, :],
                        in_=wo[c4:c4 + 4].rearrange("c p n -> p c n"))
                attg0B = po.tile([128, 8, 512], BF)
                attg1B = po.tile([128, 8, 512], BF)
                for s2 in (0, 2):
                    nc.scalar.dma_start(
                        out=attg0B[:, 2 * s2:2 * s2 + 4, :],
                        in_=a2aB_out[s2:s2 + 2].rearrange(
                            "s (c p) n -> p (s c) n", p=128))
                    nc.scalar.dma_start(
                        out=attg1B[:, 2 * s2:2 * s2 + 4, :],
                        in_=a2aB_out[4 + s2:4 + s2 + 2].rearrange(
                            "s (c p) n -> p (s c) n", p=128))
                attg = [po.tile([128, 8, 512], BF, name=f"attg_{h}")
                        for h in range(2)]
                for hf, (g0, g1) in ((0, (attg0A, attg1A)),
                                     (1, (attg0B, attg1B))):
                    for k8 in range(8):
                        t0 = po.tile([128, 512], BF, tag="blend0", bufs=3,
                                     name=f"bl0_{hf}_{k8}")
                        nc.vector.tensor_scalar_mul(t0[:], g0[:, k8, :],
                                                    bs[:, 0:1])
                        t1 = po.tile([128, 512], BF, tag="blend1", bufs=3,
                                     name=f"bl1_{hf}_{k8}")
                        nc.vector.tensor_scalar_mul(t1[:], g1[:, k8, :],
                                                    bs[:, 1:2])
                        nc.vector.tensor_tensor(out=attg[hf][:, k8, :],
                                                in0=t0[:], in1=t1[:], op=ADD)
                # half-partials: all A-half GEMM work first (ready early),
                # B-half after A2A#2 lands; sum partials at the end
                part = [[None] * 4 for _ in range(2)]
                for hf in range(2):
                    for sp in range(2):
                        psos = []
                        for si in range(2):
                            pso = pop.tile([128, HID], F32, tag="po", bufs=2,
                                           name=f"pso_{hf}_{sp}_{si}")
                            psos.append(pso)
                        for k8 in range(8):
                            k = 8 * hf + k8
                            for si in range(2):
                                st = 2 * sp + si
                                lhsT = attg[hf][:, k8,
                                                128 * st:128 * st + 128]
                                for u in range(4):
                                    nc.tensor.matmul(
                                        psos[si][:, 512 * u:512 * u + 512],
                                        lhsT=lhsT,
                                        rhs=wo_sb[:, k, 512 * u:512 * u + 512],
                                        start=(k8 == 0), stop=(k8 == 7))
                        for si in range(2):
                            st = 2 * sp + si
                            pt = po.tile([128, HID], BF, tag="part", bufs=8,
                                         name=f"part_{hf}_{st}")
                            nc.scalar.copy(pt[:], psos[si][:])
                            part[hf][st] = pt
                for st in range(4):
                    oso = po.tile([128, HID], F32, tag="oso", bufs=2,
                                  name=f"oso_{st}")
                    nc.vector.tensor_tensor(out=oso[:], in0=part[0][st][:],
                                            in1=part[1][st][:], op=ADD)
                    nc.sync.dma_start(
                        out=out_part[128 * st:128 * st + 128, :],
                        in_=oso[:])

    nc.compile()
    return nc


def _rope_tables():
    inv_freq = 1.0 / (ROPE_BASE ** (np.arange(0, HD, 2, dtype=np.float32) / HD))
    t = np.arange(S, dtype=np.float32)
    freqs = np.outer(t, inv_freq).astype(np.float32)  # [S, 32]
    cosT = np.cos(freqs).T  # [32, S]
    sinT = np.sin(freqs).T
    cos64 = np.concatenate([cosT, cosT], axis=0)          # [64, S]
    sin64 = np.concatenate([-sinT, sinT], axis=0)         # signed
    bf = ml_dtypes.bfloat16
    cos2 = np.concatenate([cos64, cos64], axis=0).astype(bf)
    sin2 = np.concatenate([sin64, sin64], axis=0).astype(bf)
    return cos2, sin2


def _np_reference(hidden_states, attention_mask, q_w, q_b, k_w, k_b, v_w, v_b,
                  o_w):
    hs = hidden_states.astype(np.float64)
    q = hs @ q_w.T.astype(np.float64) + q_b
    k = hs @ k_w.T.astype(np.float64) + k_b
    v = hs @ v_w.T.astype(np.float64) + v_b
    q = q.reshape(B, S, NH, HD).transpose(0, 2, 1, 3)
    k = k.reshape(B, S, NKV, HD).transpose(0, 2, 1, 3)
    v = v.reshape(B, S, NKV, HD).transpose(0, 2, 1, 3)
    inv_freq = 1.0 / (ROPE_BASE ** (np.arange(0, HD, 2) / HD))
    t = np.arange(S)
    freqs = np.outer(t, inv_freq)
    emb = np.concatenate([freqs, freqs], axis=-1)
    cos, sin = np.cos(emb), np.sin(emb)

    def rot(x):
        h = x.shape[-1] // 2
        return np.concatenate([-x[..., h:], x[..., :h]], axis=-1)

    q = q * cos + rot(q) * sin
    k = k * cos + rot(k) * sin
    k = np.repeat(k, GROUPS, axis=1)
    v = np.repeat(v, GROUPS, axis=1)
    sc = np.einsum("bhqd,bhkd->bhqk", q, k) / math.sqrt(HD)
    sc = sc + attention_mask.astype(np.float64)
    sc = sc - sc.max(axis=-1, keepdims=True)
    p = np.exp(sc)
    p = p / p.sum(axis=-1, keepdims=True)
    out = np.einsum("bhqk,bhkd->bhqd", p, v)
    out = out.transpose(0, 2, 1, 3).reshape(B, S, HID)
    return (out @ o_w.T.astype(np.float64)).astype(np.float32)


def _pack_chunks17(mat, bias):
    """[2048, M] weights + [M] bias -> [17, 128, M] with bias in row 0 of
    chunk 16."""
    m = mat.shape[1]
    out = np.zeros((17, 128, m), dtype=mat.dtype)
    out[:16] = mat.reshape(16, 128, m)
    out[16, 0, :] = bias
    return out


# local-head order along the a2a row axis (matches qperm pair layout)
LH = [0, 4, 1, 5, 2, 6, 3, 7]


def _make_in_maps(inputs):
    hs = np.asarray(inputs["hidden_states"], np.float32)
    mask = np.asarray(inputs["attention_mask"], np.float32)
    q_w = np.asarray(inputs["q_w"], np.float32)
    q_b = np.asarray(inputs["q_b"], np.float32)
    k_w = np.asarray(inputs["k_w"], np.float32)
    k_b = np.asarray(inputs["k_b"], np.float32)
    v_w = np.asarray(inputs["v_w"], np.float32)
    v_b = np.asarray(inputs["v_b"], np.float32)
    o_w = np.asarray(inputs["o_w"], np.float32)
    m2 = mask[0, 0]

    bf = ml_dtypes.bfloat16
    cos2, sin2 = _rope_tables()
    # binary mask for the diagonal block, transposed orientation [k, q]
    maskbin = (m2[0:128, 0:128].T == 0.0).astype(bf)

    # wo rows permuted to the (half, source, chunk) pair-order layout:
    # chunk (hf, s, c) covers global heads 8s+2hf+c and 8s+2hf+c+4
    o_wT = np.ascontiguousarray(o_w.T.astype(bf))  # [2048 in, 2048 out]
    rows = []
    for hf in range(2):
        for s in range(4):
            for c in range(2):
                g1 = 8 * s + 2 * hf + c
                rows.extend(range(64 * g1, 64 * g1 + 64))
                rows.extend(range(64 * (g1 + 4), 64 * (g1 + 4) + 64))
    wo_np = o_wT[np.array(rows)].reshape(16, 128, HID)

    hsT_packed = []
    for b in range(B):
        h = np.zeros((17, 128, S), dtype=bf)
        h[:16] = np.ascontiguousarray(hs[b].T).astype(bf).reshape(16, 128, S)
        h[16, 0, :] = 1.0
        hsT_packed.append(h)

    q_wT = np.ascontiguousarray(q_w.T).astype(bf)  # [2048, 2048]
    k_wT = np.ascontiguousarray(k_w.T).astype(bf)  # [2048, 512]
    v_wT = np.ascontiguousarray(v_w.T).astype(bf)

    # within-group q-head permutation [0,4,1,5,2,6,3,7] (pair layout)
    qperm = np.concatenate(
        [np.arange(64 * hh, 64 * hh + 64) for t in range(4)
         for hh in (t, t + 4)])

    in_maps = []
    for c in range(NCORES):
        b, g = c // 4, c % 4
        in_maps.append({
            "hsT": hsT_packed[b],
            "wq": _pack_chunks17(q_wT[:, 512 * g:512 * g + 512][:, qperm],
                                 q_b[512 * g:512 * g + 512][qperm].astype(bf)),
            "wk": _pack_chunks17(k_wT[:, 128 * g:128 * g + 128],
                                 k_b[128 * g:128 * g + 128].astype(bf)),
            "wv": _pack_chunks17(v_wT[:, 128 * g:128 * g + 128],
                                 v_b[128 * g:128 * g + 128].astype(bf)),
            "wo": wo_np,
            "cos2": cos2,
            "sin2": sin2,
            "maskbin": maskbin,
            "bsel": np.stack(
                [np.full(128, 1.0 - b, np.float32),
                 np.full(128, float(b), np.float32)], axis=1),
        })
    return in_maps


def kernel(**inputs):
    mask = np.asarray(inputs["attention_mask"], np.float32)
    m2 = mask[0, 0]
    causal_ok = bool(
        np.all(m2[np.tril_indices(S)] == 0.0)
        and np.all(m2[np.triu_indices(S, 1)] < -1e8))
    if not causal_ok:
        return _np_reference(
            np.asarray(inputs["hidden_states"], np.float32), mask,
            *(np.asarray(inputs[k], np.float32)
              for k in ("q_w", "q_b", "k_w", "k_b", "v_w", "v_b", "o_w")))

    if "nc" not in _CACHED:
        _CACHED["nc"] = _build_nc()
    nc = _CACHED["nc"]
    in_maps = _make_in_maps(inputs)

    res = run_bass_kernel_spmd(nc, in_maps, list(range(NCORES)))

    out = np.empty((B, S, HID), dtype=np.float32)
    for c in range(NCORES):
        b, g = c // 4, c % 4
        out[b, 512 * g:512 * g + 512, :] = np.asarray(
            res.results[c]["out_part"], np.float32)
    return out

